# revision 55
# baseline (speedup 1.0000x reference)
"""Trainium2 Bass kernel for FeatureTransformerSlice (embedding lookup).

out[b, :] = bias + sum_f mask(idx[b,f]) * val[b,f] * weight[max(idx[b,f],0), :]

Strategy (8 NeuronCores, data-parallel over batch):
  - Each core owns B/8 = 2048 batch rows, split into NSHARDS=2 shards of 1024
    rows.  Per shard the host remaps the used vocab ids (np.unique, ~22.6K <
    int16 max) to a compact bf16 table W[uniq] that lives in that core's HBM;
    bf16 halves the random-gather HBM traffic (1KB rows) and the 2e-2 rel-err
    budget dwarfs the 2^-9 rounding.
  - Gathers use the SWDGE dma_gather instruction: one GpSimd call fetches 512
    random table rows (4 features x 128 batch rows) into a chunk of the tile's
    [128, 32, 512] bf16 SBUF dest (flat slot i = j*128+p lands at [p, j, :]).
    8 calls/tile round-robin over 4 SWDGE queues so descriptor generation
    (~5-9ns/row of Q7 ucode, the baseline's serializing bottleneck at 512
    indirect_dma_start calls) runs in parallel rings and the 16 DMA engines
    stay ~92% busy at their ~21B/ns random-1KB-row roofline (~198us/core).
  - Per tile: the leading 24 features reduce on the PE as bf16 diag(val)
    matmuls accumulated in fp32 PSUM (bias enters via a K=1 ones x bias
    matmul); diagonals are built on DVE as val-broadcast x replicated
    identity.  The trailing 8 features are scaled on DVE with one broadcast
    multiply and binary-tree-added in bf16, then folded into PSUM with an
    identity matmul.  ACT copies PSUM to a bf16 staging tile (output is
    written bf16 and upcast to f32 on the host).
  - Indices arrive pre-wrapped from the host in the SWDGE layout (16-partition
    wrap replicated 8x across the 128 partitions); values arrive bf16 in
    feature-major [128, tiles*32] layout; masked (negative) features get val=0.
  - A tiny dummy gather up front overlaps the Q7 SWDGE library load (~14us)
    with the input DMAs.

Measured: rel err ~5e-3, ~270us HW exec (all 8 cores; ~3x the 798us baseline).
"""

import numpy as np
import ml_dtypes

bf16 = ml_dtypes.bfloat16

P = 128
B = 16384
F = 32
V = 40960
O = 512
NCORES = 8
BC = B // NCORES          # rows per core
TILES = BC // P           # batch tiles per core (16)
NI = P * F                # gathered rows per tile per call (4096)
S = NI // 16              # idx columns per tile in the 16-partition wrap (256)

# tuning knobs
NSHARDS = 2               # vocab-remap shards per core (2 -> ~22.6K uniq ids)
U_PAD = 23552             # padded compact-table rows (fits int16, > max uniq)
NSHARDS_FB = 4            # fallback if a shard overflows U_PAD
U_PAD_FB = 16384
DVE_FEATS = 8             # trailing features per tile computed off the PE
                          # (power of two; DVE broadcast-mult + binary tree of
                          # wide bf16 adds, folded into PSUM via an identity
                          # matmul)
ACT_FEATS = 0             # of DVE_FEATS, how many are scaled on ACT
G_BUFS = 3                # gather-tile double/triple buffering
IDX_PER_CALL = 1024       # rows per dma_gather call (divisor of NI, mult of 128;
                          # <= 1024 so one call's descriptors fit the 16KB
                          # SWDGE ring carveout; smaller = smoother DMA flow)
SWDGE_QUEUES = 4          # spread gather calls over SWDGE rings


def build_kernel(nshards=NSHARDS, u_pad=U_PAD, dve_feats=DVE_FEATS,
                 act_feats=ACT_FEATS, g_bufs=G_BUFS, idx_per_call=IDX_PER_CALL,
                 swdge_queues=SWDGE_QUEUES):
    import concourse.bacc as bacc
    import concourse.bass as bass
    import concourse.mybir as mybir
    import concourse.tile as tile

    f32 = mybir.dt.float32
    bf = mybir.dt.bfloat16
    i16 = mybir.dt.int16

    tiles_per_shard = TILES // nshards
    calls_per_tile = NI // idx_per_call
    j_per_call = idx_per_call // P

    nc = bacc.Bacc("TRN2", target_bir_lowering=False, debug=False,
                   num_swdge_queues=swdge_queues,
                   dynamic_dma_scratch_size=32768)

    w_ds = [nc.dram_tensor(f"w{h}", [u_pad, O], bf, kind="ExternalInput")
            for h in range(nshards)]
    ix_d = nc.dram_tensor("ix", [P, TILES * S], i16, kind="ExternalInput")
    vb_d = nc.dram_tensor("vb", [P, TILES * F], bf, kind="ExternalInput")
    b_d = nc.dram_tensor("b", [1, O], bf, kind="ExternalInput")
    id_d = nc.dram_tensor("id", [P, P], bf, kind="ExternalInput")
    mk_d = nc.dram_tensor("mk", [P, F * P], bf, kind="ExternalInput")
    out_d = nc.dram_tensor("out", [BC, O], bf, kind="ExternalOutput")

    with tile.TileContext(nc) as tc:
        with (
            tc.tile_pool(name="io", bufs=1) as io,
            tc.tile_pool(name="gp", bufs=g_bufs) as gp,
            tc.tile_pool(name="dp", bufs=2) as dp,
            tc.tile_pool(name="ob", bufs=3) as ob,
            tc.tile_pool(name="ps", bufs=4, space="PSUM") as ps,
        ):
            # ---- warm up the SWDGE ucode library with a tiny dummy gather
            # so its load overlaps the input DMAs instead of serializing in
            # front of the first real gather ----
            warm_ix = io.tile([P, 1], i16)
            nc.vector.memset(warm_ix[:], 0)
            warm_g = io.tile([P, 1, O], bf)
            nc.gpsimd.dma_gather(
                out_ap=warm_g[:], in_ap=w_ds[0].ap(), idxs_ap=warm_ix[:],
                num_idxs=16, num_idxs_reg=16, elem_size=O,
            )

            # ---- one-time loads (ix first: the gathers depend only on it) ----
            ix_sb = io.tile([P, TILES * S], i16)
            nc.sync.dma_start(out=ix_sb[:], in_=ix_d.ap())
            vb_sb = io.tile([P, TILES * F], bf)
            nc.sync.dma_start(out=vb_sb[:], in_=vb_d.ap())
            bias_sb = io.tile([1, O], bf)
            nc.sync.dma_start(out=bias_sb[:], in_=b_d.ap())
            assert dve_feats == 0 or (dve_feats >= 2
                                      and dve_feats & (dve_feats - 1) == 0)
            assert 0 <= act_feats <= dve_feats
            ones_sb = io.tile([1, P], bf)
            nc.vector.memset(ones_sb[:], 1.0)
            pe_feats = F - dve_feats
            id_sb = io.tile([P, P], bf)
            nc.sync.dma_start(out=id_sb[:], in_=id_d.ap())
            mk_sb = io.tile([P, F, P], bf)
            nc.sync.dma_start(
                out=mk_sb[:], in_=mk_d.ap().rearrange("p (f q) -> p f q", q=P))
            if act_feats:
                vf_sb = io.tile([P, TILES * F], f32)
                nc.vector.tensor_copy(out=vf_sb[:], in_=vb_sb[:])

            # ---- main loop over batch tiles ----
            for t in range(TILES):
                w_d = w_ds[t // tiles_per_shard]
                G = gp.tile([P, F, O], bf, tag="g")
                for cc in range(calls_per_tile):
                    j0 = cc * j_per_call
                    nc.gpsimd.dma_gather(
                        out_ap=G[:, j0:j0 + j_per_call, :],
                        in_ap=w_d.ap(),
                        idxs_ap=ix_sb[:, t * S + cc * (S // calls_per_tile):
                                      t * S + (cc + 1) * (S // calls_per_tile)],
                        num_idxs=idx_per_call,
                        num_idxs_reg=idx_per_call,
                        elem_size=O,
                        queue_num=(1 + t * calls_per_tile + cc) % swdge_queues,
                    )

                # PE owns the leading pe_feats features (consumed forward, as
                # chunks arrive); DVE owns the trailing ones.
                d = dp.tile([P, pe_feats, P], bf, tag="d")
                vb_pe = vb_sb[:, t * F:t * F + pe_feats].unsqueeze(2)
                nc.vector.tensor_tensor(
                    out=d[:], in0=vb_pe.to_broadcast([P, pe_feats, P]),
                    in1=mk_sb[:, :pe_feats, :], op=mybir.AluOpType.mult,
                )

                acc = None
                if dve_feats:
                    sc = dp.tile([P, dve_feats, O], bf, tag="s")
                    nv = dve_feats - act_feats
                    if nv:
                        vb_dv = vb_sb[:, t * F + pe_feats:
                                      t * F + pe_feats + nv].unsqueeze(2)
                        nc.vector.tensor_tensor(
                            out=sc[:, :nv, :],
                            in0=vb_dv.to_broadcast([P, nv, O]),
                            in1=G[:, pe_feats:pe_feats + nv, :],
                            op=mybir.AluOpType.mult,
                        )
                    for a in range(act_feats):
                        j = pe_feats + nv + a
                        nc.scalar.activation(
                            out=sc[:, nv + a, :], in_=G[:, j, :],
                            func=mybir.ActivationFunctionType.Copy,
                            scale=vf_sb[:, t * F + j:t * F + j + 1],
                        )
                    cur, width, lvl = sc, dve_feats, 0
                    while width > 1:
                        half = width // 2
                        nxt = dp.tile([P, half, O], bf, tag=f"t{lvl}")
                        nc.vector.tensor_tensor(
                            out=nxt[:], in0=cur[:, :half, :],
                            in1=cur[:, half:width, :], op=mybir.AluOpType.add,
                        )
                        cur, width, lvl = nxt, half, lvl + 1
                    acc = cur

                psum = ps.tile([P, O], f32)
                nc.tensor.matmul(
                    out=psum[:], lhsT=ones_sb[:], rhs=bias_sb[:],
                    start=True, stop=False,
                )
                for j in range(pe_feats):
                    nc.tensor.matmul(
                        out=psum[:], lhsT=d[:, j, :], rhs=G[:, j, :],
                        start=False, stop=acc is None and j == pe_feats - 1,
                    )
                if acc is not None:
                    nc.tensor.matmul(
                        out=psum[:], lhsT=id_sb[:], rhs=acc[:, 0, :],
                        start=False, stop=True,
                    )

                out_sb = ob.tile([P, O], bf, tag="o")
                nc.scalar.activation(
                    out=out_sb[:], in_=psum[:],
                    func=mybir.ActivationFunctionType.Copy,
                )
                nc.sync.dma_start(
                    out=out_d.ap()[t * P:(t + 1) * P, :], in_=out_sb[:],
                )

    nc.compile()
    return nc


_nc_cache = {}


def _get_nc(nshards, u_pad):
    key = (nshards, u_pad)
    if key not in _nc_cache:
        _nc_cache[key] = build_kernel(nshards=nshards, u_pad=u_pad)
    return _nc_cache[key]


def _pack_shard(idx_eff, val_eff, w_bf, u_pad):
    """Remap one shard's indices to a compact table.

    Returns (padded bf16 table [u_pad, O], wrapped int16 idx [tiles, P, S],
    bf16 val [P, tiles*F]) or None if the shard's vocab overflows u_pad."""
    rows = idx_eff.shape[0]
    tiles = rows // P
    uniq, inv = np.unique(idx_eff, return_inverse=True)
    if len(uniq) > min(u_pad, 32768):
        return None
    table = np.zeros((u_pad, O), bf16)
    table[:len(uniq)] = w_bf[uniq]
    idx16 = inv.reshape(rows, F).astype(np.int16)
    a = idx16.reshape(tiles, P, F).transpose(0, 2, 1)   # [t, j, p]: flat i=j*128+p
    a = a.reshape(tiles, NI).reshape(tiles, S, 16)      # [t, s, p16]: i=s*16+p16
    a = a.transpose(0, 2, 1)                            # [t, p16, s]
    ix = np.broadcast_to(a[:, None], (tiles, 8, 16, S)).reshape(tiles, P, S)
    v = val_eff.reshape(tiles, P, F).transpose(1, 0, 2).reshape(P, tiles * F)
    return table, np.ascontiguousarray(ix), v.astype(bf16)


def _prep_in_maps(feature_indices, feature_values, weight, bias):
    fi = np.asarray(feature_indices)
    fv = np.asarray(feature_values, dtype=np.float32)
    w_bf = np.asarray(weight, dtype=np.float32).astype(bf16)
    b_bf = np.asarray(bias, dtype=np.float32).astype(bf16).reshape(1, O)

    mask = fi >= 0
    val_eff = np.where(mask, fv, np.float32(0.0))
    idx_eff = np.where(mask, fi, 0).astype(np.int64)

    for nshards, u_pad in [(NSHARDS, U_PAD), (NSHARDS_FB, U_PAD_FB)]:
        ss = BC // nshards
        in_maps = []
        ok = True
        for c in range(NCORES):
            tables, ixs, vbs = [], [], []
            for h in range(nshards):
                lo = c * BC + h * ss
                packed = _pack_shard(idx_eff[lo:lo + ss], val_eff[lo:lo + ss],
                                     w_bf, u_pad)
                if packed is None:
                    ok = False
                    break
                tables.append(packed[0])
                ixs.append(packed[1])
                vbs.append(packed[2])
            if not ok:
                break
            m = {f"w{h}": tables[h] for h in range(nshards)}
            m["ix"] = np.concatenate(ixs, 0).transpose(1, 0, 2).reshape(
                P, TILES * S).copy()
            m["vb"] = np.concatenate(vbs, 1)
            m["b"] = b_bf
            m["id"] = np.eye(P, dtype=bf16)
            m["mk"] = np.tile(np.eye(P, dtype=bf16)[:, None, :],
                              (1, F, 1)).reshape(P, F * P)
            in_maps.append(m)
        if ok:
            return nshards, u_pad, in_maps
    raise RuntimeError("vocab shard overflowed even the fallback split")


def _ensure_ntff_hook():
    """The agent image lacks antenv.axon_hooks; synthesize it (best effort) so
    a trace=True run (or a stray BASS_TRACE=1 env) never crashes on import."""
    import sys
    import types
    if "antenv.axon_hooks" in sys.modules:
        return
    try:
        from trn_agent_boot.trn_boot import _ntff_profile_via_ctypes
        hook = _ntff_profile_via_ctypes("/opt/axon/libaxon_pjrt.so")
    except Exception:
        hook = None
    try:
        mod = types.ModuleType("antenv.axon_hooks")
        mod.get_axon_ntff_profile_hook = lambda: hook
        mod.set_axon_ntff_profile_hook = lambda h: None
        sys.modules["antenv.axon_hooks"] = mod
        import antenv
        antenv.axon_hooks = mod
    except Exception:
        pass
    try:
        from concourse import bass_utils
        bass_utils.upload_artifacts = lambda tmpdir: tmpdir  # no S3 in sandbox
    except Exception:
        pass


def run_on_hw(feature_indices, feature_values, weight, bias, trace=False):
    from concourse import bass_utils
    _ensure_ntff_hook()
    nshards, u_pad, in_maps = _prep_in_maps(
        feature_indices, feature_values, weight, bias)
    nc = _get_nc(nshards, u_pad)
    res = bass_utils.run_bass_kernel_spmd(
        nc, in_maps, core_ids=list(range(NCORES)), trace=trace,
    )
    out = np.concatenate([r["out"] for r in res.results], axis=0)
    return out.astype(np.float32), res


def kernel(feature_indices, feature_values, weight, bias):
    out, _ = run_on_hw(feature_indices, feature_values, weight, bias,
                       trace=False)
    return out


# revision 56
# speedup vs baseline: 1.0656x; 1.0656x over previous
"""Trainium2 Bass kernel for FeatureTransformerSlice (embedding lookup).

out[b, :] = bias + sum_f mask(idx[b,f]) * val[b,f] * weight[max(idx[b,f],0), :]

Strategy (8 NeuronCores, data-parallel over batch):
  - Each core owns B/8 = 2048 batch rows, split into NSHARDS=2 shards of 1024
    rows.  Per shard the host remaps the used vocab ids (np.unique, ~22.6K <
    int16 max) to a compact bf16 table W[uniq] that lives in that core's HBM;
    bf16 halves the random-gather HBM traffic (1KB rows) and the 2e-2 rel-err
    budget dwarfs the 2^-9 rounding.
  - Gathers use the SWDGE dma_gather instruction: one GpSimd call fetches 512
    random table rows (4 features x 128 batch rows) into a chunk of the tile's
    [128, 32, 512] bf16 SBUF dest (flat slot i = j*128+p lands at [p, j, :]).
    8 calls/tile round-robin over 4 SWDGE queues so descriptor generation
    (~5-9ns/row of Q7 ucode, the baseline's serializing bottleneck at 512
    indirect_dma_start calls) runs in parallel rings and the 16 DMA engines
    stay ~92% busy at their ~21B/ns random-1KB-row roofline (~198us/core).
  - Per tile: the leading 24 features reduce on the PE as bf16 diag(val)
    matmuls accumulated in fp32 PSUM (bias enters via a K=1 ones x bias
    matmul); diagonals are built on DVE as val-broadcast x replicated
    identity.  The trailing 8 features are scaled on DVE with one broadcast
    multiply and binary-tree-added in bf16, then folded into PSUM with an
    identity matmul.  ACT copies PSUM to a bf16 staging tile (output is
    written bf16 and upcast to f32 on the host).
  - Indices arrive pre-wrapped from the host in the SWDGE layout (16-partition
    wrap replicated 8x across the 128 partitions); values arrive bf16 in
    feature-major [128, tiles*32] layout; masked (negative) features get val=0.
  - A tiny dummy gather up front overlaps the Q7 SWDGE library load (~14us)
    with the input DMAs.

Measured: rel err ~5e-3, ~270us HW exec (all 8 cores; ~3x the 798us baseline).
"""

import numpy as np
import ml_dtypes

bf16 = ml_dtypes.bfloat16

P = 128
B = 16384
F = 32
V = 40960
O = 512
NCORES = 8
BC = B // NCORES          # rows per core
TILES = BC // P           # batch tiles per core (16)
NI = P * F                # gathered rows per tile per call (4096)
S = NI // 16              # idx columns per tile in the 16-partition wrap (256)

# tuning knobs
NSHARDS = 2               # vocab-remap shards per core (2 -> ~22.6K uniq ids)
U_PAD = 23552             # padded compact-table rows (fits int16, > max uniq)
NSHARDS_FB = 4            # fallback if a shard overflows U_PAD
U_PAD_FB = 16384
DVE_FEATS = 8             # trailing features per tile computed off the PE
                          # (power of two; DVE broadcast-mult + binary tree of
                          # wide bf16 adds, folded into PSUM via an identity
                          # matmul)
ACT_FEATS = 0             # of DVE_FEATS, how many are scaled on ACT
G_BUFS = 3                # gather-tile double/triple buffering
IDX_PER_CALL = 1024       # rows per dma_gather call (divisor of NI, mult of 128;
                          # <= 1024 so one call's descriptors fit the 16KB
                          # SWDGE ring carveout; smaller = smoother DMA flow)
SWDGE_QUEUES = 4          # spread gather calls over SWDGE rings


def build_kernel(nshards=NSHARDS, u_pad=U_PAD, dve_feats=DVE_FEATS,
                 act_feats=ACT_FEATS, g_bufs=G_BUFS, idx_per_call=IDX_PER_CALL,
                 swdge_queues=SWDGE_QUEUES):
    import concourse.bacc as bacc
    import concourse.bass as bass
    import concourse.mybir as mybir
    import concourse.tile as tile

    f32 = mybir.dt.float32
    bf = mybir.dt.bfloat16
    i16 = mybir.dt.int16

    tiles_per_shard = TILES // nshards
    calls_per_tile = NI // idx_per_call
    j_per_call = idx_per_call // P

    nc = bacc.Bacc("TRN2", target_bir_lowering=False, debug=False,
                   num_swdge_queues=swdge_queues)

    w_ds = [nc.dram_tensor(f"w{h}", [u_pad, O], bf, kind="ExternalInput")
            for h in range(nshards)]
    ix_d = nc.dram_tensor("ix", [P, TILES * S], i16, kind="ExternalInput")
    vb_d = nc.dram_tensor("vb", [P, TILES * F], bf, kind="ExternalInput")
    b_d = nc.dram_tensor("b", [1, O], bf, kind="ExternalInput")
    id_d = nc.dram_tensor("id", [P, P], bf, kind="ExternalInput")
    mk_d = nc.dram_tensor("mk", [P, F * P], bf, kind="ExternalInput")
    out_d = nc.dram_tensor("out", [BC, O], bf, kind="ExternalOutput")

    with tile.TileContext(nc) as tc:
        with (
            tc.tile_pool(name="io", bufs=1) as io,
            tc.tile_pool(name="gp", bufs=g_bufs) as gp,
            tc.tile_pool(name="dp", bufs=2) as dp,
            tc.tile_pool(name="ob", bufs=3) as ob,
            tc.tile_pool(name="ps", bufs=4, space="PSUM") as ps,
        ):
            # ---- warm up the SWDGE ucode library with a tiny dummy gather
            # so its load overlaps the input DMAs instead of serializing in
            # front of the first real gather ----
            warm_ix = io.tile([P, 1], i16)
            nc.vector.memset(warm_ix[:], 0)
            warm_g = io.tile([P, 1, O], bf)
            nc.gpsimd.dma_gather(
                out_ap=warm_g[:], in_ap=w_ds[0].ap(), idxs_ap=warm_ix[:],
                num_idxs=16, num_idxs_reg=16, elem_size=O,
            )

            # ---- one-time loads (ix first: the gathers depend only on it) ----
            ix_sb = io.tile([P, TILES * S], i16)
            nc.sync.dma_start(out=ix_sb[:], in_=ix_d.ap())
            vb_sb = io.tile([P, TILES * F], bf)
            nc.sync.dma_start(out=vb_sb[:], in_=vb_d.ap())
            bias_sb = io.tile([1, O], bf)
            nc.sync.dma_start(out=bias_sb[:], in_=b_d.ap())
            assert dve_feats == 0 or (dve_feats >= 2
                                      and dve_feats & (dve_feats - 1) == 0)
            assert 0 <= act_feats <= dve_feats
            ones_sb = io.tile([1, P], bf)
            nc.vector.memset(ones_sb[:], 1.0)
            pe_feats = F - dve_feats
            id_sb = io.tile([P, P], bf)
            nc.sync.dma_start(out=id_sb[:], in_=id_d.ap())
            mk_sb = io.tile([P, F, P], bf)
            nc.sync.dma_start(
                out=mk_sb[:], in_=mk_d.ap().rearrange("p (f q) -> p f q", q=P))
            if act_feats:
                vf_sb = io.tile([P, TILES * F], f32)
                nc.vector.tensor_copy(out=vf_sb[:], in_=vb_sb[:])

            # ---- main loop over batch tiles ----
            for t in range(TILES):
                w_d = w_ds[t // tiles_per_shard]
                G = gp.tile([P, F, O], bf, tag="g")
                for cc in range(calls_per_tile):
                    j0 = cc * j_per_call
                    nc.gpsimd.dma_gather(
                        out_ap=G[:, j0:j0 + j_per_call, :],
                        in_ap=w_d.ap(),
                        idxs_ap=ix_sb[:, t * S + cc * (S // calls_per_tile):
                                      t * S + (cc + 1) * (S // calls_per_tile)],
                        num_idxs=idx_per_call,
                        num_idxs_reg=idx_per_call,
                        elem_size=O,
                        queue_num=(1 + t * calls_per_tile + cc) % swdge_queues,
                    )

                # PE owns the leading pe_feats features (consumed forward, as
                # chunks arrive); DVE owns the trailing ones.
                d = dp.tile([P, pe_feats, P], bf, tag="d")
                vb_pe = vb_sb[:, t * F:t * F + pe_feats].unsqueeze(2)
                nc.vector.tensor_tensor(
                    out=d[:], in0=vb_pe.to_broadcast([P, pe_feats, P]),
                    in1=mk_sb[:, :pe_feats, :], op=mybir.AluOpType.mult,
                )

                acc = None
                if dve_feats:
                    sc = dp.tile([P, dve_feats, O], bf, tag="s")
                    nv = dve_feats - act_feats
                    if nv:
                        vb_dv = vb_sb[:, t * F + pe_feats:
                                      t * F + pe_feats + nv].unsqueeze(2)
                        nc.vector.tensor_tensor(
                            out=sc[:, :nv, :],
                            in0=vb_dv.to_broadcast([P, nv, O]),
                            in1=G[:, pe_feats:pe_feats + nv, :],
                            op=mybir.AluOpType.mult,
                        )
                    for a in range(act_feats):
                        j = pe_feats + nv + a
                        nc.scalar.activation(
                            out=sc[:, nv + a, :], in_=G[:, j, :],
                            func=mybir.ActivationFunctionType.Copy,
                            scale=vf_sb[:, t * F + j:t * F + j + 1],
                        )
                    cur, width, lvl = sc, dve_feats, 0
                    while width > 1:
                        half = width // 2
                        nxt = dp.tile([P, half, O], bf, tag=f"t{lvl}")
                        nc.vector.tensor_tensor(
                            out=nxt[:], in0=cur[:, :half, :],
                            in1=cur[:, half:width, :], op=mybir.AluOpType.add,
                        )
                        cur, width, lvl = nxt, half, lvl + 1
                    acc = cur

                psum = ps.tile([P, O], f32)
                nc.tensor.matmul(
                    out=psum[:], lhsT=ones_sb[:], rhs=bias_sb[:],
                    start=True, stop=False,
                )
                for j in range(pe_feats):
                    nc.tensor.matmul(
                        out=psum[:], lhsT=d[:, j, :], rhs=G[:, j, :],
                        start=False, stop=acc is None and j == pe_feats - 1,
                    )
                if acc is not None:
                    nc.tensor.matmul(
                        out=psum[:], lhsT=id_sb[:], rhs=acc[:, 0, :],
                        start=False, stop=True,
                    )

                out_sb = ob.tile([P, O], bf, tag="o")
                nc.scalar.activation(
                    out=out_sb[:], in_=psum[:],
                    func=mybir.ActivationFunctionType.Copy,
                )
                nc.sync.dma_start(
                    out=out_d.ap()[t * P:(t + 1) * P, :], in_=out_sb[:],
                )

    nc.compile()
    return nc


_nc_cache = {}


def _get_nc(nshards, u_pad):
    key = (nshards, u_pad)
    if key not in _nc_cache:
        _nc_cache[key] = build_kernel(nshards=nshards, u_pad=u_pad)
    return _nc_cache[key]


def _pack_shard(idx_eff, val_eff, w_bf, u_pad):
    """Remap one shard's indices to a compact table.

    Returns (padded bf16 table [u_pad, O], wrapped int16 idx [tiles, P, S],
    bf16 val [P, tiles*F]) or None if the shard's vocab overflows u_pad."""
    rows = idx_eff.shape[0]
    tiles = rows // P
    uniq, inv = np.unique(idx_eff, return_inverse=True)
    if len(uniq) > min(u_pad, 32768):
        return None
    table = np.zeros((u_pad, O), bf16)
    table[:len(uniq)] = w_bf[uniq]
    idx16 = inv.reshape(rows, F).astype(np.int16)
    a = idx16.reshape(tiles, P, F).transpose(0, 2, 1)   # [t, j, p]: flat i=j*128+p
    a = a.reshape(tiles, NI).reshape(tiles, S, 16)      # [t, s, p16]: i=s*16+p16
    a = a.transpose(0, 2, 1)                            # [t, p16, s]
    ix = np.broadcast_to(a[:, None], (tiles, 8, 16, S)).reshape(tiles, P, S)
    v = val_eff.reshape(tiles, P, F).transpose(1, 0, 2).reshape(P, tiles * F)
    return table, np.ascontiguousarray(ix), v.astype(bf16)


def _prep_in_maps(feature_indices, feature_values, weight, bias):
    fi = np.asarray(feature_indices)
    fv = np.asarray(feature_values, dtype=np.float32)
    w_bf = np.asarray(weight, dtype=np.float32).astype(bf16)
    b_bf = np.asarray(bias, dtype=np.float32).astype(bf16).reshape(1, O)

    mask = fi >= 0
    val_eff = np.where(mask, fv, np.float32(0.0))
    idx_eff = np.where(mask, fi, 0).astype(np.int64)

    for nshards, u_pad in [(NSHARDS, U_PAD), (NSHARDS_FB, U_PAD_FB)]:
        ss = BC // nshards
        in_maps = []
        ok = True
        for c in range(NCORES):
            tables, ixs, vbs = [], [], []
            for h in range(nshards):
                lo = c * BC + h * ss
                packed = _pack_shard(idx_eff[lo:lo + ss], val_eff[lo:lo + ss],
                                     w_bf, u_pad)
                if packed is None:
                    ok = False
                    break
                tables.append(packed[0])
                ixs.append(packed[1])
                vbs.append(packed[2])
            if not ok:
                break
            m = {f"w{h}": tables[h] for h in range(nshards)}
            m["ix"] = np.concatenate(ixs, 0).transpose(1, 0, 2).reshape(
                P, TILES * S).copy()
            m["vb"] = np.concatenate(vbs, 1)
            m["b"] = b_bf
            m["id"] = np.eye(P, dtype=bf16)
            m["mk"] = np.tile(np.eye(P, dtype=bf16)[:, None, :],
                              (1, F, 1)).reshape(P, F * P)
            in_maps.append(m)
        if ok:
            return nshards, u_pad, in_maps
    raise RuntimeError("vocab shard overflowed even the fallback split")


def _ensure_ntff_hook():
    """The agent image lacks antenv.axon_hooks; synthesize it (best effort) so
    a trace=True run (or a stray BASS_TRACE=1 env) never crashes on import."""
    import sys
    import types
    if "antenv.axon_hooks" in sys.modules:
        return
    try:
        from trn_agent_boot.trn_boot import _ntff_profile_via_ctypes
        hook = _ntff_profile_via_ctypes("/opt/axon/libaxon_pjrt.so")
    except Exception:
        hook = None
    try:
        mod = types.ModuleType("antenv.axon_hooks")
        mod.get_axon_ntff_profile_hook = lambda: hook
        mod.set_axon_ntff_profile_hook = lambda h: None
        sys.modules["antenv.axon_hooks"] = mod
        import antenv
        antenv.axon_hooks = mod
    except Exception:
        pass
    try:
        from concourse import bass_utils
        bass_utils.upload_artifacts = lambda tmpdir: tmpdir  # no S3 in sandbox
    except Exception:
        pass


def run_on_hw(feature_indices, feature_values, weight, bias, trace=False):
    from concourse import bass_utils
    _ensure_ntff_hook()
    nshards, u_pad, in_maps = _prep_in_maps(
        feature_indices, feature_values, weight, bias)
    nc = _get_nc(nshards, u_pad)
    res = bass_utils.run_bass_kernel_spmd(
        nc, in_maps, core_ids=list(range(NCORES)), trace=trace,
    )
    out = np.concatenate([r["out"] for r in res.results], axis=0)
    return out.astype(np.float32), res


def kernel(feature_indices, feature_values, weight, bias):
    out, _ = run_on_hw(feature_indices, feature_values, weight, bias,
                       trace=False)
    return out
